# revision 12
# baseline (speedup 1.0000x reference)
"""BotRGCN (2x RGCNConv + MLPs) on 8 Trainium2 NeuronCores.

Strategy: shard aggregation by destination node (each core owns 12500 dst
nodes); replicate the cheap input MLP so the x0 gather table needs no
collective. Per RGCN layer, each core:
  - dma_gathers x[src] rows (fp16, 256B) from a node-feature table,
  - aggregates with TensorE: psum[f, d] += sum_e xg[e, f] * sel[e, d]
    (128-edge batches; sel = one-hot(dst_local % 128) * 1/cnt built on DVE),
  - slot order is (relation, superblock-pair, window, block): one PSUM tile
    accumulates a (r, sb) superblock across all 4 source windows, drained
    ONCE per (r, sb) [scalar engine] and transformed by W_rel[r] [TensorE]
    into the aggT accumulator [DVE add],
  - gather calls cover a whole (r, sb-pair, window) range (~2k edges) to
    amortize the gpsimd SWDGE fixed cost (~1us/call).
Layer-2's table is distributed with a single AllGather; its W_root transform
is emitted first so TensorE works during the collective.
PSUM note: zero regions are 2048B banks — exactly ONE matmul start/stop per
bank, or re-arming pending-zero clobbers sibling blocks' partial sums.
Table rows are permuted within each 1024-node tile (row = p*8 + j) so the
transposed table writes are 2KB-contiguous per partition.
"""
import numpy as np

import concourse.bacc as bacc
import concourse.mybir as mybir
import concourse.tile as tile
from concourse.bass_utils import run_bass_kernel_spmd

# ---------------- problem constants (hardcoded per the task contract) ----
N, E, R, D = 100000, 1600000, 5, 128
C = 8                     # cores
NSLAB = 12500             # real nodes owned per core
NBLK = 104                # 128-dst blocks per core (padded)
SLAB = NBLK * 128         # 13312 padded slab rows per core
NPAD = C * SLAB           # 106496 table rows
NTILE = NPAD // 1024      # 104 1024-row table tiles
W = 4                     # source windows (int16 gather index limit)
WIN = NPAD // W           # 26624 rows per window (< 32768)
SB = 8                    # blocks per psum superblock
NSB = NBLK // SB          # 13
GSB = 2                   # superblocks per gather/psum group
NG = (NSB + GSB - 1) // GSB   # 7 groups
GBLK = GSB * SB           # 16 blocks per group
CALL_MAX = 3072           # max edges per dma_gather call (24 batches)
F16 = mybir.dt.float16
F32 = mybir.dt.float32
I16 = mybir.dt.int16

_AluOp = mybir.AluOpType
_Act = mybir.ActivationFunctionType


# ---------------- walrus workaround --------------------------------------
def _split_sync_waits(nc, maxw=1):
    """walrus build here rejects >1 sync wait per instruction; hoist excess
    waits onto same-engine InstDrain instructions inserted just before."""
    n_split = 0
    for fn in nc.m.functions:
        for bb in fn.blocks:
            new_insts = []
            for inst in bb.instructions:
                si = inst.sync_info
                if si is not None and si.on_wait and len(si.on_wait) > maxw:
                    waits = list(si.on_wait)
                    excess, keep = waits[:-maxw], waits[-maxw:]
                    for i in range(0, len(excess), maxw):
                        d = mybir.InstNoOp(name=f"waitsplit_{n_split}", ins=[], outs=[])
                        n_split += 1
                        d.engine = inst.engine
                        d.sync_info = mybir.SyncInfo(
                            on_wait=excess[i:i + maxw], on_update=[])
                        nc.register_instruction(d)
                        new_insts.append(d)
                    si.on_wait = keep
                new_insts.append(inst)
            bb.instructions[:] = new_insts
    return n_split


# ---------------- host-side prep ------------------------------------------
def _wrap_idx(a):
    """[C, TOT] int16 -> [C, 128, TOT//16]: slot i at (i%16, i//16), x8 replicated."""
    Cc, TOT = a.shape
    t = a.reshape(Cc, TOT // 16, 16).transpose(0, 2, 1)
    return np.ascontiguousarray(np.tile(t, (1, 8, 1)))


def _table_row(node):
    """node id -> permuted table row. Within each 1024-node tile, row =
    p*8 + j (p = node%128 partition, j = block-of-128 within the tile), so
    transposed [128p, 8j, 128f] tile writes are 2KB-contiguous per p."""
    c = node // NSLAB
    i = node - c * NSLAB
    sb = i >> 10
    j = (i >> 7) & 7
    p = i & 127
    return c * SLAB + sb * 1024 + p * 8 + j


def _host_prep(edge_index, edge_type):
    src = np.asarray(edge_index[0], dtype=np.int64)
    dst = np.asarray(edge_index[1], dtype=np.int64)
    et = np.asarray(edge_type, dtype=np.int64)

    owner = dst // NSLAB
    dloc = dst - owner * NSLAB
    blk = dloc >> 7
    srow = _table_row(src)
    wi = srow // WIN
    widx = (srow - wi * WIN).astype(np.int16)

    cnt_full = np.bincount(et * N + dst, minlength=R * N)
    dl7 = (dloc & 127).astype(np.float32)
    inv = (1.0 / np.maximum(cnt_full[et * N + dst], 1)).astype(np.float32)

    counts = np.zeros((C, R, W, NBLK), np.int64)
    np.add.at(counts, (owner, et, wi, blk), 1)
    B = np.maximum(1, -(-counts.max(axis=0) // 128))  # [R, W, NBLK] batches

    # slot order: (r, g=blk//GBLK, w, blk, k). base offsets + schedule + calls.
    base = np.zeros((R, W, NBLK), np.int64)
    sched = []
    groups = []   # [(r, g, [call dicts])]
    r_gb0 = []    # first global batch index per relation
    gb = 0
    for r in range(R):
        r_gb0.append(gb)
        for g in range(NG):
            blks = range(GBLK * g, min(GBLK * (g + 1), NBLK))
            grp_calls = []
            for w in range(W):
                call_gb0 = gb
                for b in blks:
                    base[r, w, b] = gb * 128
                    nb = int(B[r, w, b])
                    for k in range(nb):
                        # PSUM zero regions are 2048B banks (4 blocks): issue
                        # exactly ONE start/stop per bank, or re-arming the
                        # bank's pending-zero clobbers sibling blocks' sums.
                        sched.append({
                            "r": r, "w": w, "blk": b, "sb": b >> 3, "bi": b & 7,
                            "start": (w == 0 and k == 0 and (b & 3) == 0),
                            "stop": (w == W - 1 and k == nb - 1 and (b & 3) == 3),
                        })
                        gb += 1
                # split the (r,g,w) range into <=CALL_MAX-slot calls
                nb_rw = gb - call_gb0
                done = 0
                while done < nb_rw:
                    nb_c = min(nb_rw - done, CALL_MAX // 128)
                    grp_calls.append({"w": w, "gb0": call_gb0 + done, "nb": nb_c})
                    done += nb_c
            groups.append((r, g, grp_calls))
    r_gb0.append(gb)
    TOT = gb * 128
    assert len(sched) * 128 == TOT

    # slot position assignment (ranks within each (owner, r, g, w, blk) group)
    key = (((owner * R + et) * NG + blk // GBLK) * W + wi) * NBLK + blk
    order = np.lexsort((srow, key))
    ks = key[order]
    grp_start = np.r_[0, np.flatnonzero(np.diff(ks)) + 1]
    grp_len = np.diff(np.r_[grp_start, E])
    ranks = np.arange(E) - np.repeat(grp_start, grp_len)
    pos = base[et[order], wi[order], blk[order]] + ranks

    xidx = np.zeros((C, TOT), np.int16)
    mdl = np.zeros((C, TOT), np.float32)
    minv = np.zeros((C, TOT), np.float32)
    xidx[owner[order], pos] = widx[order]
    mdl[owner[order], pos] = dl7[order]
    minv[owner[order], pos] = inv[order]

    nb_all = TOT // 128
    meta_dl = mdl.reshape(C, nb_all, 128).transpose(0, 2, 1)   # [C,128,NB]
    meta_inv = minv.reshape(C, nb_all, 128).transpose(0, 2, 1)
    return xidx, (np.ascontiguousarray(meta_dl), np.ascontiguousarray(meta_inv)), \
        sched, groups, r_gb0, TOT


# ---------------- device program ------------------------------------------
def _build(sched, groups, r_gb0, TOT):
    nc = bacc.Bacc("TRN2", target_bir_lowering=False, debug=False,
                   num_devices=C, num_swdge_queues=4)
    TOT16 = TOT // 16

    # inputs
    featT = nc.dram_tensor("featT", [18, NPAD], F16, kind="ExternalInput")
    featT_own = nc.dram_tensor("featT_own", [18, SLAB], F16, kind="ExternalInput")
    xidx_d = nc.dram_tensor("xidx", [128, TOT16], I16, kind="ExternalInput")
    NB_ALL = TOT // 128
    mdl_d = nc.dram_tensor("meta_dl", [128, NB_ALL], F32, kind="ExternalInput")
    minv_d = nc.dram_tensor("meta_inv", [128, NB_ALL], F32, kind="ExternalInput")
    iota_d = nc.dram_tensor("iota128", [128, 1024], F16, kind="ExternalInput")
    wnc_d = nc.dram_tensor("wnc", [18, 128], F16, kind="ExternalInput")
    win_d = nc.dram_tensor("win", [128, 128], F16, kind="ExternalInput")
    wrel_d = nc.dram_tensor("wrel", [R * 128, 128], F16, kind="ExternalInput")
    wroot_d = nc.dram_tensor("wroot", [128, 128], F16, kind="ExternalInput")
    wo1_d = nc.dram_tensor("wo1", [128, 128], F16, kind="ExternalInput")
    wo2_d = nc.dram_tensor("wo2", [128, 2], F16, kind="ExternalInput")
    bias_d = nc.dram_tensor("biases", [128, 5], F32, kind="ExternalInput")
    ident_d = nc.dram_tensor("ident", [128, 128], F16, kind="ExternalInput")
    out_d = nc.dram_tensor("out", [2, SLAB], F32, kind="ExternalOutput")

    with tile.TileContext(nc) as tc:
        with (
            tc.tile_pool(name="const", bufs=1) as constp,
            tc.tile_pool(name="slabs", bufs=1) as slabp,
            tc.tile_pool(name="dram", bufs=1, space="DRAM") as dramp,
        ):
            # constants to SBUF
            wnc = constp.tile([18, 128], F16)
            nc.sync.dma_start(wnc[:], wnc_d[:])
            win = constp.tile([128, 128], F16)
            nc.sync.dma_start(win[:], win_d[:])
            wrel = [constp.tile([128, 128], F16, name=f"wrel{r}") for r in range(R)]
            for r in range(R):
                nc.sync.dma_start(wrel[r][:], wrel_d[r * 128:(r + 1) * 128, :])
            wroot = constp.tile([128, 128], F16)
            nc.sync.dma_start(wroot[:], wroot_d[:])
            wo1 = constp.tile([128, 128], F16)
            nc.sync.dma_start(wo1[:], wo1_d[:])
            wo2 = constp.tile([128, 2], F16)
            nc.sync.dma_start(wo2[:], wo2_d[:])
            biases = constp.tile([128, 5], F32)
            nc.sync.dma_start(biases[:], bias_d[:])
            ident = constp.tile([128, 128], F16)
            nc.sync.dma_start(ident[:], ident_d[:])
            iota128 = constp.tile([128, 1024], F16)
            nc.sync.dma_start(iota128[:], iota_d[:])
            meta_dl = constp.tile([128, NB_ALL], F32, name="meta_dl")
            nc.sync.dma_start(meta_dl[:], mdl_d[:])
            meta_inv = constp.tile([128, NB_ALL], F32, name="meta_inv")
            nc.sync.dma_start(meta_inv[:], minv_d[:])
            b_in = biases[:, 0:1]
            b_rgcn = biases[:, 1:2]
            b_o2 = biases[0:2, 3:4]
            b_o1p = biases[:, 4:5]

            # resident slabs (feature-major fp16)
            xT_A = slabp.tile([128, SLAB], F16, name="xT_A")   # x0T own slab
            xT_B = slabp.tile([128, SLAB], F16, name="xT_B")   # x1T own slab
            aggT = slabp.tile([128, SLAB], F16, name="aggT")

            # x1slab rows follow the permuted table convention: row p*8+j in
            # each 1024-node tile holds node j*128+p -> shape [t][p][j][f].
            x1slab = dramp.tile([NSB, 128, SB, D], F16, name="x1slab")
            x0tab = dramp.tile([NPAD, D], F16, name="x0tab")
            x1tab = dramp.tile([NPAD, D], F16, name="x1tab", addr_space="Shared")
            x0tab3 = x0tab[:].rearrange("(t p j) f -> t p (j f)", p=128, j=SB)

            # -------- phase 0a: own-slab MLP -> xT_A (feature-major) --------
            # -------- phase 0b: replicated full-table MLP -> x0tab ----------
            with (
                tc.tile_pool(name="p0", bufs=3) as p0,
                tc.tile_pool(name="ps0", bufs=3, space="PSUM") as ps0,
                tc.tile_pool(name="tps0", bufs=2, space="PSUM") as tps0,
            ):
                def mlp_tile(cs, dst_ap, src=featT):
                    ft = p0.tile([18, 1024], F16, tag="ft")
                    nc.sync.dma_start(ft[:], src[:, cs])
                    pa = ps0.tile([128, 1024], F32, tag="ps")
                    for j in range(2):
                        nc.tensor.matmul(pa[:, j * 512:(j + 1) * 512], wnc[:],
                                         ft[:, j * 512:(j + 1) * 512], start=True, stop=True)
                    xnc = p0.tile([128, 1024], F16, tag="xnc")
                    # bias folded into wnc row 17; Lrelu on scalar engine
                    nc.scalar.activation(xnc[:], pa[:], _Act.Lrelu, alpha=0.01)
                    pb = ps0.tile([128, 1024], F32, tag="ps")
                    for j in range(2):
                        nc.tensor.matmul(pb[:, j * 512:(j + 1) * 512], win[:],
                                         xnc[:, j * 512:(j + 1) * 512], start=True, stop=True)
                    nc.scalar.activation(dst_ap, pb[:], _Act.Lrelu,
                                         bias=b_in, alpha=0.01)

                for t in range(NSB):    # own slab, feature-major
                    cs = slice(t * 1024, (t + 1) * 1024)
                    mlp_tile(cs, xT_A[:, cs], src=featT_own)
                for t in range(NTILE):  # full table, transposed + written out
                    cs = slice(t * 1024, (t + 1) * 1024)
                    xt = p0.tile([128, 1024], F16, tag="xt")
                    mlp_tile(cs, xt[:])
                    tp = tps0.tile([128, 1024], F16, tag="tp")
                    for j in range(8):
                        js = slice(j * 128, (j + 1) * 128)
                        nc.tensor.transpose(tp[:, js], xt[:, js], ident[:])
                    st = p0.tile([128, 1024], F16, tag="st")
                    nc.scalar.activation(st[:], tp[:], _Act.Copy)
                    nc.sync.dma_start(x0tab3[t], st[:])

            def transpose_out(src_slab, dst_dram4):
                """src [128, SLAB] feature-major -> dst [NSB,128,SB,D] permuted
                node-major (row p*8+j within each 1024-node tile)."""
                with (
                    tc.tile_pool(name="tr", bufs=3) as trp,
                    tc.tile_pool(name="trps", bufs=3, space="PSUM") as trps,
                ):
                    for t in range(NSB):
                        tp = trps.tile([128, 1024], F16, tag="tp")
                        for j in range(8):
                            bs = slice(t * 1024 + j * 128, t * 1024 + (j + 1) * 128)
                            nc.tensor.transpose(tp[:, j * 128:(j + 1) * 128],
                                                src_slab[:, bs], ident[:])
                        st = trp.tile([128, 1024], F16, tag="st")
                        nc.scalar.activation(st[:], tp[:], _Act.Copy)
                        nc.sync.dma_start(dst_dram4[t], st[:])

            # ---------------- RGCN layer -----------------------------------
            def rgcn_layer(tab, xT_prev, xT_next, finalize=True):
                with (
                    tc.tile_pool(name="gidx", bufs=2) as gip,
                    tc.tile_pool(name="gdat", bufs=6) as gdp,
                    tc.tile_pool(name="selp", bufs=8) as selp,
                    tc.tile_pool(name="stg", bufs=4) as stgp,
                    tc.tile_pool(name="mps", bufs=3, space="PSUM") as mps,
                    tc.tile_pool(name="tps", bufs=1, space="PSUM") as tps,
                ):
                    # W_root first: TensorE fills aggT while gathers stream
                    # (for layer 2 this overlaps the AllGather).
                    for s in range(NSB):
                        tp = tps.tile([128, 1024], F32, tag="tp")
                        for j in range(2):
                            osl = slice(s * 1024 + j * 512, s * 1024 + (j + 1) * 512)
                            nc.tensor.matmul(tp[:, j * 512:(j + 1) * 512], wroot[:],
                                             xT_prev[:, osl], start=True, stop=True)
                        osl = slice(s * 1024, (s + 1) * 1024)
                        nc.vector.tensor_copy(aggT[:, osl], tp[:])

                    mp_tiles = {}
                    pending = None
                    qn = 0
                    xi_r = None
                    cur_r = -1

                    def flush(r, sbs):
                        for sb in sbs:
                            mp = mp_tiles.pop(sb)
                            stg = stgp.tile([128, 1024], F16, tag="stg")
                            nc.scalar.activation(stg[:], mp[:], _Act.Copy)
                            tp = tps.tile([128, 1024], F32, tag="tp")
                            for j in range(2):
                                js = slice(j * 512, (j + 1) * 512)
                                nc.tensor.matmul(tp[:, js], wrel[r][:], stg[:, js],
                                                 start=True, stop=True)
                            osl = slice(sb * 1024, (sb + 1) * 1024)
                            nc.vector.tensor_add(aggT[:, osl], aggT[:, osl], tp[:])

                    for r, g, grp_calls in groups:
                        if r != cur_r:
                            cur_r = r
                            rcols = (r_gb0[r + 1] - r_gb0[r]) * 8
                            xi_r = gip.tile([128, rcols], I16, tag="xi")
                            nc.sync.dma_start(
                                xi_r[:], xidx_d[:, r_gb0[r] * 8:r_gb0[r] * 8 + rcols])
                        for call in grp_calls:
                            w, gb0, nb = call["w"], call["gb0"], call["nb"]
                            ns = nb * 128
                            c0 = (gb0 - r_gb0[r]) * 8
                            xg = gdp.tile([128, nb, 128], F16, tag="xg")
                            nc.gpsimd.dma_gather(
                                xg[:], tab[w * WIN:(w + 1) * WIN, :],
                                xi_r[:, c0:c0 + ns // 16], ns, ns, D,
                                single_packet=False, queue_num=qn)
                            qn = (qn + 1) % 4
                            for i in range(nb):
                                m = sched[gb0 + i]
                                sb = m["sb"]
                                if sb not in mp_tiles:
                                    mp_tiles[sb] = mps.tile(
                                        [128, 1024], F32, tag="mp", name=f"mp_{r}_{g}_{sb}")
                                mp = mp_tiles[sb]
                                sel = selp.tile([128, 128], F16, tag="sel")
                                nc.vector.tensor_scalar(
                                    sel[:], iota128[:, :128],
                                    meta_dl[:, gb0 + i:gb0 + i + 1],
                                    meta_inv[:, gb0 + i:gb0 + i + 1],
                                    op0=_AluOp.is_equal, op1=_AluOp.mult)
                                nc.tensor.matmul(
                                    mp[:, m["bi"] * 128:(m["bi"] + 1) * 128],
                                    xg[:, i, :], sel[:],
                                    start=m["start"], stop=m["stop"])
                        if pending is not None:
                            flush(*pending)
                        pending = (r, sorted({s >> 3 for s in
                                              range(GBLK * g, min(GBLK * (g + 1), NBLK))}))
                    flush(*pending)
                    if finalize:
                        for s in range(NSB):
                            osl = slice(s * 1024, (s + 1) * 1024)
                            nc.scalar.activation(
                                xT_next[:, osl], aggT[:, osl], _Act.Identity, bias=b_rgcn)

            rgcn_layer(x0tab, xT_A, xT_B)
            transpose_out(xT_B, x1slab)
            nc.gpsimd.collective_compute(
                "AllGather", _AluOp.bypass, ins=[x1slab.opt()], outs=[x1tab.opt()],
                replica_groups=[list(range(C))])
            rgcn_layer(x1tab, xT_B, xT_A, finalize=False)  # x2 stays in aggT

            # ---------------- final MLP ------------------------------------
            with (
                tc.tile_pool(name="pf", bufs=3) as pf,
                tc.tile_pool(name="psf", bufs=2, space="PSUM") as psf,
            ):
                for t in range(NSB):
                    cs = slice(t * 1024, (t + 1) * 1024)
                    pa = psf.tile([128, 1024], F32, tag="fa")
                    for j in range(2):
                        js = slice(t * 1024 + j * 512, t * 1024 + (j + 1) * 512)
                        nc.tensor.matmul(pa[:, j * 512:(j + 1) * 512],
                                         wo1[:], aggT[:, js], start=True, stop=True)
                    o1 = pf.tile([128, 1024], F16, tag="fo1")
                    nc.scalar.activation(o1[:], pa[:], _Act.Lrelu,
                                         bias=b_o1p, alpha=0.01)
                    pb = psf.tile([2, 1024], F32, tag="fb")
                    for j in range(2):
                        nc.tensor.matmul(pb[:, j * 512:(j + 1) * 512], wo2[:],
                                         o1[:, j * 512:(j + 1) * 512], start=True, stop=True)
                    ot = pf.tile([2, 1024], F32, tag="fot")
                    nc.scalar.activation(ot[:], pb[:], _Act.Identity, bias=b_o2)
                    nc.sync.dma_start(out_d[:, cs], ot[:])

    nc.compile()
    _split_sync_waits(nc)
    return nc


def prepare(inputs):
    """Build (nc, in_maps) for the SPMD run — shared by kernel() and bench."""
    num_prop = np.asarray(inputs["num_prop"], np.float32)
    cat_prop = np.asarray(inputs["cat_prop"], np.float32)
    edge_index = np.asarray(inputs["edge_index"])
    edge_type = np.asarray(inputs["edge_type"])

    xidx, (meta_dl, meta_inv), sched, groups, r_gb0, TOT = _host_prep(
        edge_index, edge_type)
    nc = _build(sched, groups, r_gb0, TOT)

    # featT full table, feature-major, CANONICAL column order (column
    # c*SLAB + i = features of core c's local node i); the device writes
    # x0tab rows in the permuted order itself.
    feat = np.concatenate([num_prop, cat_prop], axis=1)          # [N, 17]
    featF = np.zeros((NPAD, 17), np.float16)
    nodes = np.arange(N)
    featF[(nodes // NSLAB) * SLAB + nodes % NSLAB] = feat.astype(np.float16)
    featT_full = np.ascontiguousarray(np.concatenate(
        [featF.T, np.ones((1, NPAD), np.float16)], axis=0))      # [18, NPAD]

    wnp = np.asarray(inputs["W_np"], np.float32)
    wcp = np.asarray(inputs["W_cp"], np.float32)
    bnp = np.asarray(inputs["b_np"], np.float32)
    bcp = np.asarray(inputs["b_cp"], np.float32)
    wnc = np.zeros((18, 128), np.float16)
    wnc[0:6, 0:64] = wnp
    wnc[6:17, 64:128] = wcp
    wnc[17, 0:64] = bnp
    wnc[17, 64:128] = bcp

    biases = np.zeros((128, 5), np.float32)
    biases[:, 0] = np.asarray(inputs["b_in"], np.float32)
    biases[:, 1] = np.asarray(inputs["b_rgcn"], np.float32)
    biases[:, 2] = np.asarray(inputs["b_o1"], np.float32)
    biases[0:2, 3] = np.asarray(inputs["b_o2"], np.float32)
    # final MLP reads aggT (pre-bias x2): fold b_rgcn through W_o1
    biases[:, 4] = (np.asarray(inputs["b_o1"], np.float32)
                    + np.asarray(inputs["b_rgcn"], np.float32)
                    @ np.asarray(inputs["W_o1"], np.float32))

    common = {
        "iota128": np.tile(np.arange(1024, dtype=np.float16), (128, 1)),
        "wnc": wnc,
        "win": np.asarray(inputs["W_in"], np.float16),
        "wrel": np.asarray(inputs["W_rel"], np.float16).reshape(R * 128, 128),
        "wroot": np.asarray(inputs["W_root"], np.float16),
        "wo1": np.asarray(inputs["W_o1"], np.float16),
        "wo2": np.asarray(inputs["W_o2"], np.float16),
        "biases": biases,
        "ident": np.eye(128, dtype=np.float16),
    }
    xw = _wrap_idx(xidx)
    in_maps = []
    for c in range(C):
        m = dict(common)
        m["featT"] = featT_full
        m["featT_own"] = np.ascontiguousarray(
            featT_full[:, c * SLAB:(c + 1) * SLAB])
        m["xidx"] = xw[c]
        m["meta_dl"] = meta_dl[c]
        m["meta_inv"] = meta_inv[c]
        in_maps.append(m)
    return nc, in_maps


def kernel(**inputs) -> np.ndarray:
    nc, in_maps = prepare(inputs)
    res = run_bass_kernel_spmd(nc, in_maps, list(range(C)))
    out = np.concatenate(
        [res.results[c]["out"][:, :NSLAB].T for c in range(C)], axis=0)
    return out.astype(np.float32)


# revision 15
# speedup vs baseline: 2076.1606x; 2076.1606x over previous
"""BotRGCN (2x RGCNConv + MLPs) on 8 Trainium2 NeuronCores.

Strategy: shard aggregation by destination node (each core owns 12500 dst
nodes); replicate the cheap input MLP so the x0 gather table needs no
collective. Per RGCN layer, each core:
  - dma_gathers x[src] rows (fp16, 256B) from a node-feature table,
  - aggregates with TensorE: psum[f, d] += sum_e xg[e, f] * sel[e, d]
    (128-edge batches; sel = one-hot(dst_local % 128) * 1/cnt built on DVE),
  - slot order is (relation, superblock-pair, window, block): one PSUM tile
    accumulates a (r, sb) superblock across all 4 source windows, drained
    ONCE per (r, sb) [scalar engine] and transformed by W_rel[r] [TensorE]
    into the aggT accumulator [DVE add],
  - gather calls cover a whole (r, sb-pair, window) range (~2k edges) to
    amortize the gpsimd SWDGE fixed cost (~1us/call).
Layer-2's table is distributed with a single AllGather; its W_root transform
is emitted first so TensorE works during the collective.
PSUM note: zero regions are 2048B banks — exactly ONE matmul start/stop per
bank, or re-arming pending-zero clobbers sibling blocks' partial sums.
Table rows are permuted within each 1024-node tile (row = p*8 + j) so the
transposed table writes are 2KB-contiguous per partition.
"""
import numpy as np

import concourse.bacc as bacc
import concourse.mybir as mybir
import concourse.tile as tile
from concourse.bass_utils import run_bass_kernel_spmd

# ---------------- problem constants (hardcoded per the task contract) ----
N, E, R, D = 100000, 1600000, 5, 128
C = 8                     # cores
NSLAB = 12500             # real nodes owned per core
NBLK = 104                # 128-dst blocks per core (padded)
SLAB = NBLK * 128         # 13312 padded slab rows per core
NPAD = C * SLAB           # 106496 table rows
NTILE = NPAD // 1024      # 104 1024-row table tiles
W = 4                     # source windows (int16 gather index limit)
WIN = NPAD // W           # 26624 rows per window (< 32768)
SB = 8                    # blocks per psum superblock
NSB = NBLK // SB          # 13
GSB = 2                   # superblocks per gather/psum group
NG = (NSB + GSB - 1) // GSB   # 7 groups
GBLK = GSB * SB           # 16 blocks per group
CALL_MAX = 3072           # max edges per dma_gather call (24 batches)
F16 = mybir.dt.float16
F32 = mybir.dt.float32
I16 = mybir.dt.int16

_AluOp = mybir.AluOpType
_Act = mybir.ActivationFunctionType


# ---------------- walrus workaround --------------------------------------
def _split_sync_waits(nc, maxw=1):
    """walrus build here rejects >1 sync wait per instruction; hoist excess
    waits onto same-engine InstDrain instructions inserted just before."""
    n_split = 0
    for fn in nc.m.functions:
        for bb in fn.blocks:
            new_insts = []
            for inst in bb.instructions:
                si = inst.sync_info
                if si is not None and si.on_wait and len(si.on_wait) > maxw:
                    waits = list(si.on_wait)
                    excess, keep = waits[:-maxw], waits[-maxw:]
                    for i in range(0, len(excess), maxw):
                        d = mybir.InstNoOp(name=f"waitsplit_{n_split}", ins=[], outs=[])
                        n_split += 1
                        d.engine = inst.engine
                        d.sync_info = mybir.SyncInfo(
                            on_wait=excess[i:i + maxw], on_update=[])
                        nc.register_instruction(d)
                        new_insts.append(d)
                    si.on_wait = keep
                new_insts.append(inst)
            bb.instructions[:] = new_insts
    return n_split


# ---------------- host-side prep ------------------------------------------
def _wrap_idx(a):
    """[C, TOT] int16 -> [C, 128, TOT//16]: slot i at (i%16, i//16), x8 replicated."""
    Cc, TOT = a.shape
    t = a.reshape(Cc, TOT // 16, 16).transpose(0, 2, 1)
    return np.ascontiguousarray(np.tile(t, (1, 8, 1)))


def _table_row(node):
    """node id -> permuted table row. Within each 1024-node tile, row =
    p*8 + j (p = node%128 partition, j = block-of-128 within the tile), so
    transposed [128p, 8j, 128f] tile writes are 2KB-contiguous per p."""
    c = node // NSLAB
    i = node - c * NSLAB
    sb = i >> 10
    j = (i >> 7) & 7
    p = i & 127
    return c * SLAB + sb * 1024 + p * 8 + j


def _host_prep(edge_index, edge_type):
    src = np.asarray(edge_index[0], dtype=np.int64)
    dst = np.asarray(edge_index[1], dtype=np.int64)
    et = np.asarray(edge_type, dtype=np.int64)

    owner = dst // NSLAB
    dloc = dst - owner * NSLAB
    blk = dloc >> 7
    srow = _table_row(src)
    wi = srow // WIN
    widx = (srow - wi * WIN).astype(np.int16)

    cnt_full = np.bincount(et * N + dst, minlength=R * N)
    dl7 = (dloc & 127).astype(np.float32)
    inv = (1.0 / np.maximum(cnt_full[et * N + dst], 1)).astype(np.float32)

    counts = np.zeros((C, R, W, NBLK), np.int64)
    np.add.at(counts, (owner, et, wi, blk), 1)
    B = np.maximum(1, -(-counts.max(axis=0) // 128))  # [R, W, NBLK] batches

    # slot order: (r, g=blk//GBLK, w, blk, k). base offsets + schedule + calls.
    base = np.zeros((R, W, NBLK), np.int64)
    sched = []
    groups = []   # [(r, g, [call dicts])]
    r_gb0 = []    # first global batch index per relation
    gb = 0
    for r in range(R):
        r_gb0.append(gb)
        for g in range(NG):
            blks = range(GBLK * g, min(GBLK * (g + 1), NBLK))
            grp_calls = []
            for w in range(W):
                call_gb0 = gb
                for b in blks:
                    base[r, w, b] = gb * 128
                    nb = int(B[r, w, b])
                    for k in range(nb):
                        # PSUM zero regions are 2048B banks (4 blocks): issue
                        # exactly ONE start/stop per bank, or re-arming the
                        # bank's pending-zero clobbers sibling blocks' sums.
                        sched.append({
                            "r": r, "w": w, "blk": b, "sb": b >> 3, "bi": b & 7,
                            "start": (w == 0 and k == 0 and (b & 3) == 0),
                            "stop": (w == W - 1 and k == nb - 1 and (b & 3) == 3),
                        })
                        gb += 1
                # split the (r,g,w) range into <=CALL_MAX-slot calls
                nb_rw = gb - call_gb0
                done = 0
                while done < nb_rw:
                    nb_c = min(nb_rw - done, CALL_MAX // 128)
                    grp_calls.append({"w": w, "gb0": call_gb0 + done, "nb": nb_c})
                    done += nb_c
            groups.append((r, g, grp_calls))
    r_gb0.append(gb)
    TOT = gb * 128
    assert len(sched) * 128 == TOT

    # slot position assignment (ranks within each (owner, r, g, w, blk) group)
    key = (((owner * R + et) * NG + blk // GBLK) * W + wi) * NBLK + blk
    order = np.lexsort((srow, key))
    ks = key[order]
    grp_start = np.r_[0, np.flatnonzero(np.diff(ks)) + 1]
    grp_len = np.diff(np.r_[grp_start, E])
    ranks = np.arange(E) - np.repeat(grp_start, grp_len)
    pos = base[et[order], wi[order], blk[order]] + ranks

    xidx = np.zeros((C, TOT), np.int16)
    mdl = np.zeros((C, TOT), np.float32)
    minv = np.zeros((C, TOT), np.float32)
    xidx[owner[order], pos] = widx[order]
    mdl[owner[order], pos] = dl7[order]
    minv[owner[order], pos] = inv[order]

    nb_all = TOT // 128
    meta_dl = mdl.reshape(C, nb_all, 128).transpose(0, 2, 1)   # [C,128,NB]
    meta_inv = minv.reshape(C, nb_all, 128).transpose(0, 2, 1)
    return xidx, (np.ascontiguousarray(meta_dl), np.ascontiguousarray(meta_inv)), \
        sched, groups, r_gb0, TOT


# ---------------- device program ------------------------------------------
def _build(sched, groups, r_gb0, TOT):
    nc = bacc.Bacc("TRN2", target_bir_lowering=False, debug=False,
                   num_devices=C, num_swdge_queues=4)
    TOT16 = TOT // 16

    # inputs
    featT = nc.dram_tensor("featT", [18, NPAD], F16, kind="ExternalInput")
    featT_own = nc.dram_tensor("featT_own", [18, SLAB], F16, kind="ExternalInput")
    xidx_d = nc.dram_tensor("xidx", [128, TOT16], I16, kind="ExternalInput")
    NB_ALL = TOT // 128
    mdl_d = nc.dram_tensor("meta_dl", [128, NB_ALL], F32, kind="ExternalInput")
    minv_d = nc.dram_tensor("meta_inv", [128, NB_ALL], F32, kind="ExternalInput")
    iota_d = nc.dram_tensor("iota128", [128, 1024], F16, kind="ExternalInput")
    wnc_d = nc.dram_tensor("wnc", [18, 128], F16, kind="ExternalInput")
    win_d = nc.dram_tensor("win", [128, 128], F16, kind="ExternalInput")
    wrel_d = nc.dram_tensor("wrel", [R * 128, 128], F16, kind="ExternalInput")
    wroot_d = nc.dram_tensor("wroot", [128, 128], F16, kind="ExternalInput")
    wo1_d = nc.dram_tensor("wo1", [128, 128], F16, kind="ExternalInput")
    wo2_d = nc.dram_tensor("wo2", [128, 2], F16, kind="ExternalInput")
    bias_d = nc.dram_tensor("biases", [128, 5], F32, kind="ExternalInput")
    ident_d = nc.dram_tensor("ident", [128, 128], F16, kind="ExternalInput")
    out_d = nc.dram_tensor("out", [2, SLAB], F32, kind="ExternalOutput")

    with tile.TileContext(nc) as tc:
        with (
            tc.tile_pool(name="const", bufs=1) as constp,
            tc.tile_pool(name="slabs", bufs=1) as slabp,
            tc.tile_pool(name="dram", bufs=1, space="DRAM") as dramp,
        ):
            # constants to SBUF
            wnc = constp.tile([18, 128], F16)
            nc.sync.dma_start(wnc[:], wnc_d[:])
            win = constp.tile([128, 128], F16)
            nc.sync.dma_start(win[:], win_d[:])
            wrel = [constp.tile([128, 128], F16, name=f"wrel{r}") for r in range(R)]
            for r in range(R):
                nc.sync.dma_start(wrel[r][:], wrel_d[r * 128:(r + 1) * 128, :])
            wroot = constp.tile([128, 128], F16)
            nc.sync.dma_start(wroot[:], wroot_d[:])
            wo1 = constp.tile([128, 128], F16)
            nc.sync.dma_start(wo1[:], wo1_d[:])
            wo2 = constp.tile([128, 2], F16)
            nc.sync.dma_start(wo2[:], wo2_d[:])
            biases = constp.tile([128, 5], F32)
            nc.sync.dma_start(biases[:], bias_d[:])
            ident = constp.tile([128, 128], F16)
            nc.sync.dma_start(ident[:], ident_d[:])
            iota128 = constp.tile([128, 1024], F16)
            nc.sync.dma_start(iota128[:], iota_d[:])
            meta_dl = constp.tile([128, NB_ALL], F32, name="meta_dl")
            nc.sync.dma_start(meta_dl[:], mdl_d[:])
            meta_inv = constp.tile([128, NB_ALL], F32, name="meta_inv")
            nc.sync.dma_start(meta_inv[:], minv_d[:])
            b_in = biases[:, 0:1]
            b_rgcn = biases[:, 1:2]
            b_o2 = biases[0:2, 3:4]
            b_o1p = biases[:, 4:5]

            # resident slabs (feature-major fp16)
            xT_A = slabp.tile([128, SLAB], F16, name="xT_A")   # x0T own slab
            xT_B = slabp.tile([128, SLAB], F16, name="xT_B")   # x1T own slab
            aggT = slabp.tile([128, SLAB], F16, name="aggT")

            # x1slab rows follow the permuted table convention: row p*8+j in
            # each 1024-node tile holds node j*128+p -> shape [t][p][j][f].
            x1slab = dramp.tile([NSB, 128, SB, D], F16, name="x1slab")
            x0tab = dramp.tile([NPAD, D], F16, name="x0tab")
            x1tab = dramp.tile([NPAD, D], F16, name="x1tab", addr_space="Shared")
            x0tab3 = x0tab[:].rearrange("(t p j) f -> t p (j f)", p=128, j=SB)

            # -------- phase 0a: own-slab MLP -> xT_A (feature-major) --------
            # -------- phase 0b: replicated full-table MLP -> x0tab ----------
            with (
                tc.tile_pool(name="p0", bufs=3) as p0,
                tc.tile_pool(name="ps0", bufs=3, space="PSUM") as ps0,
                tc.tile_pool(name="tps0", bufs=2, space="PSUM") as tps0,
            ):
                def mlp_tile(cs, dst_ap, src=featT):
                    ft = p0.tile([18, 1024], F16, tag="ft")
                    nc.sync.dma_start(ft[:], src[:, cs])
                    pa = ps0.tile([128, 1024], F32, tag="ps")
                    for j in range(2):
                        nc.tensor.matmul(pa[:, j * 512:(j + 1) * 512], wnc[:],
                                         ft[:, j * 512:(j + 1) * 512], start=True, stop=True)
                    xnc = p0.tile([128, 1024], F16, tag="xnc")
                    # bias is folded into wnc row 17; Lrelu on scalar engine
                    # (a DVE max(0.01x, x) would need two PSUM reads - illegal)
                    nc.scalar.activation(xnc[:], pa[:], _Act.Lrelu, alpha=0.01)
                    pb = ps0.tile([128, 1024], F32, tag="ps")
                    for j in range(2):
                        nc.tensor.matmul(pb[:, j * 512:(j + 1) * 512], win[:],
                                         xnc[:, j * 512:(j + 1) * 512], start=True, stop=True)
                    nc.scalar.activation(dst_ap, pb[:], _Act.Lrelu,
                                         bias=b_in, alpha=0.01)

                for t in range(NSB):    # own slab, feature-major
                    cs = slice(t * 1024, (t + 1) * 1024)
                    mlp_tile(cs, xT_A[:, cs], src=featT_own)
                for t in range(NTILE):  # full table, transposed + written out
                    cs = slice(t * 1024, (t + 1) * 1024)
                    xt = p0.tile([128, 1024], F16, tag="xt")
                    mlp_tile(cs, xt[:])
                    tp = tps0.tile([128, 1024], F16, tag="tp")
                    for j in range(8):
                        js = slice(j * 128, (j + 1) * 128)
                        nc.tensor.transpose(tp[:, js], xt[:, js], ident[:])
                    st = p0.tile([128, 1024], F16, tag="st")
                    nc.vector.tensor_copy(st[:], tp[:])
                    nc.sync.dma_start(x0tab3[t], st[:])

            def transpose_out(src_slab, dst_dram4):
                """src [128, SLAB] feature-major -> dst [NSB,128,SB,D] permuted
                node-major (row p*8+j within each 1024-node tile)."""
                with (
                    tc.tile_pool(name="tr", bufs=3) as trp,
                    tc.tile_pool(name="trps", bufs=3, space="PSUM") as trps,
                ):
                    for t in range(NSB):
                        tp = trps.tile([128, 1024], F16, tag="tp")
                        for j in range(8):
                            bs = slice(t * 1024 + j * 128, t * 1024 + (j + 1) * 128)
                            nc.tensor.transpose(tp[:, j * 128:(j + 1) * 128],
                                                src_slab[:, bs], ident[:])
                        st = trp.tile([128, 1024], F16, tag="st")
                        nc.scalar.activation(st[:], tp[:], _Act.Copy)
                        nc.sync.dma_start(dst_dram4[t], st[:])

            # ---------------- RGCN layer -----------------------------------
            def rgcn_layer(tab, xT_prev, xT_next, finalize=True):
                with (
                    tc.tile_pool(name="gidx", bufs=2) as gip,
                    tc.tile_pool(name="gdat", bufs=6) as gdp,
                    tc.tile_pool(name="selp", bufs=8) as selp,
                    tc.tile_pool(name="stg", bufs=4) as stgp,
                    tc.tile_pool(name="mps", bufs=3, space="PSUM") as mps,
                    tc.tile_pool(name="tps", bufs=1, space="PSUM") as tps,
                ):
                    # W_root first: TensorE fills aggT while gathers stream
                    # (for layer 2 this overlaps the AllGather).
                    for s in range(NSB):
                        tp = tps.tile([128, 1024], F32, tag="tp")
                        for j in range(2):
                            osl = slice(s * 1024 + j * 512, s * 1024 + (j + 1) * 512)
                            nc.tensor.matmul(tp[:, j * 512:(j + 1) * 512], wroot[:],
                                             xT_prev[:, osl], start=True, stop=True)
                        osl = slice(s * 1024, (s + 1) * 1024)
                        nc.vector.tensor_copy(aggT[:, osl], tp[:])

                    mp_tiles = {}
                    pending = None
                    qn = 0
                    xi_r = None
                    cur_r = -1

                    def flush(r, sbs):
                        for sb in sbs:
                            mp = mp_tiles.pop(sb)
                            stg = stgp.tile([128, 1024], F16, tag="stg")
                            nc.scalar.activation(stg[:], mp[:], _Act.Copy)
                            tp = tps.tile([128, 1024], F32, tag="tp")
                            for j in range(2):
                                js = slice(j * 512, (j + 1) * 512)
                                nc.tensor.matmul(tp[:, js], wrel[r][:], stg[:, js],
                                                 start=True, stop=True)
                            osl = slice(sb * 1024, (sb + 1) * 1024)
                            nc.vector.tensor_add(aggT[:, osl], aggT[:, osl], tp[:])

                    for r, g, grp_calls in groups:
                        if r != cur_r:
                            cur_r = r
                            rcols = (r_gb0[r + 1] - r_gb0[r]) * 8
                            xi_r = gip.tile([128, rcols], I16, tag="xi")
                            nc.sync.dma_start(
                                xi_r[:], xidx_d[:, r_gb0[r] * 8:r_gb0[r] * 8 + rcols])
                        for call in grp_calls:
                            w, gb0, nb = call["w"], call["gb0"], call["nb"]
                            ns = nb * 128
                            c0 = (gb0 - r_gb0[r]) * 8
                            xg = gdp.tile([128, nb, 128], F16, tag="xg")
                            nc.gpsimd.dma_gather(
                                xg[:], tab[w * WIN:(w + 1) * WIN, :],
                                xi_r[:, c0:c0 + ns // 16], ns, ns, D,
                                single_packet=False, queue_num=qn)
                            qn = (qn + 1) % 4
                            for i in range(nb):
                                m = sched[gb0 + i]
                                sb = m["sb"]
                                if sb not in mp_tiles:
                                    mp_tiles[sb] = mps.tile(
                                        [128, 1024], F32, tag="mp", name=f"mp_{r}_{g}_{sb}")
                                mp = mp_tiles[sb]
                                sel = selp.tile([128, 128], F16, tag="sel")
                                nc.vector.tensor_scalar(
                                    sel[:], iota128[:, :128],
                                    meta_dl[:, gb0 + i:gb0 + i + 1],
                                    meta_inv[:, gb0 + i:gb0 + i + 1],
                                    op0=_AluOp.is_equal, op1=_AluOp.mult)
                                nc.tensor.matmul(
                                    mp[:, m["bi"] * 128:(m["bi"] + 1) * 128],
                                    xg[:, i, :], sel[:],
                                    start=m["start"], stop=m["stop"])
                        if pending is not None:
                            flush(*pending)
                        pending = (r, sorted({s >> 3 for s in
                                              range(GBLK * g, min(GBLK * (g + 1), NBLK))}))
                    flush(*pending)
                    if finalize:
                        for s in range(NSB):
                            osl = slice(s * 1024, (s + 1) * 1024)
                            nc.scalar.activation(
                                xT_next[:, osl], aggT[:, osl], _Act.Identity, bias=b_rgcn)

            rgcn_layer(x0tab, xT_A, xT_B)
            transpose_out(xT_B, x1slab)
            nc.gpsimd.collective_compute(
                "AllGather", _AluOp.bypass, ins=[x1slab.opt()], outs=[x1tab.opt()],
                replica_groups=[list(range(C))])
            rgcn_layer(x1tab, xT_B, xT_A, finalize=False)  # x2 stays in aggT

            # ---------------- final MLP ------------------------------------
            with (
                tc.tile_pool(name="pf", bufs=3) as pf,
                tc.tile_pool(name="psf", bufs=2, space="PSUM") as psf,
            ):
                for t in range(NSB):
                    cs = slice(t * 1024, (t + 1) * 1024)
                    pa = psf.tile([128, 1024], F32, tag="fa")
                    for j in range(2):
                        js = slice(t * 1024 + j * 512, t * 1024 + (j + 1) * 512)
                        nc.tensor.matmul(pa[:, j * 512:(j + 1) * 512],
                                         wo1[:], aggT[:, js], start=True, stop=True)
                    o1 = pf.tile([128, 1024], F16, tag="fo1")
                    nc.scalar.activation(o1[:], pa[:], _Act.Lrelu,
                                         bias=b_o1p, alpha=0.01)
                    pb = psf.tile([2, 1024], F32, tag="fb")
                    for j in range(2):
                        nc.tensor.matmul(pb[:, j * 512:(j + 1) * 512], wo2[:],
                                         o1[:, j * 512:(j + 1) * 512], start=True, stop=True)
                    ot = pf.tile([2, 1024], F32, tag="fot")
                    nc.scalar.activation(ot[:], pb[:], _Act.Identity, bias=b_o2)
                    nc.sync.dma_start(out_d[:, cs], ot[:])

    nc.compile()
    _split_sync_waits(nc)
    return nc


def prepare(inputs):
    """Build (nc, in_maps) for the SPMD run — shared by kernel() and bench."""
    num_prop = np.asarray(inputs["num_prop"], np.float32)
    cat_prop = np.asarray(inputs["cat_prop"], np.float32)
    edge_index = np.asarray(inputs["edge_index"])
    edge_type = np.asarray(inputs["edge_type"])

    xidx, (meta_dl, meta_inv), sched, groups, r_gb0, TOT = _host_prep(
        edge_index, edge_type)
    nc = _build(sched, groups, r_gb0, TOT)

    # featT full table, feature-major, CANONICAL column order (column
    # c*SLAB + i = features of core c's local node i); the device writes
    # x0tab rows in the permuted order itself.
    feat = np.concatenate([num_prop, cat_prop], axis=1)          # [N, 17]
    featF = np.zeros((NPAD, 17), np.float16)
    nodes = np.arange(N)
    featF[(nodes // NSLAB) * SLAB + nodes % NSLAB] = feat.astype(np.float16)
    featT_full = np.ascontiguousarray(np.concatenate(
        [featF.T, np.ones((1, NPAD), np.float16)], axis=0))      # [18, NPAD]

    wnp = np.asarray(inputs["W_np"], np.float32)
    wcp = np.asarray(inputs["W_cp"], np.float32)
    bnp = np.asarray(inputs["b_np"], np.float32)
    bcp = np.asarray(inputs["b_cp"], np.float32)
    wnc = np.zeros((18, 128), np.float16)
    wnc[0:6, 0:64] = wnp
    wnc[6:17, 64:128] = wcp
    wnc[17, 0:64] = bnp
    wnc[17, 64:128] = bcp

    biases = np.zeros((128, 5), np.float32)
    biases[:, 0] = np.asarray(inputs["b_in"], np.float32)
    biases[:, 1] = np.asarray(inputs["b_rgcn"], np.float32)
    biases[:, 2] = np.asarray(inputs["b_o1"], np.float32)
    biases[0:2, 3] = np.asarray(inputs["b_o2"], np.float32)
    # final MLP reads aggT (pre-bias x2): fold b_rgcn through W_o1
    biases[:, 4] = (np.asarray(inputs["b_o1"], np.float32)
                    + np.asarray(inputs["b_rgcn"], np.float32)
                    @ np.asarray(inputs["W_o1"], np.float32))

    common = {
        "iota128": np.tile(np.arange(1024, dtype=np.float16), (128, 1)),
        "wnc": wnc,
        "win": np.asarray(inputs["W_in"], np.float16),
        "wrel": np.asarray(inputs["W_rel"], np.float16).reshape(R * 128, 128),
        "wroot": np.asarray(inputs["W_root"], np.float16),
        "wo1": np.asarray(inputs["W_o1"], np.float16),
        "wo2": np.asarray(inputs["W_o2"], np.float16),
        "biases": biases,
        "ident": np.eye(128, dtype=np.float16),
    }
    xw = _wrap_idx(xidx)
    in_maps = []
    for c in range(C):
        m = dict(common)
        m["featT"] = featT_full
        m["featT_own"] = np.ascontiguousarray(
            featT_full[:, c * SLAB:(c + 1) * SLAB])
        m["xidx"] = xw[c]
        m["meta_dl"] = meta_dl[c]
        m["meta_inv"] = meta_inv[c]
        in_maps.append(m)
    return nc, in_maps


def kernel(**inputs) -> np.ndarray:
    nc, in_maps = prepare(inputs)
    res = run_bass_kernel_spmd(nc, in_maps, list(range(C)))
    out = np.concatenate(
        [res.results[c]["out"][:, :NSLAB].T for c in range(C)], axis=0)
    return out.astype(np.float32)
